# revision 18
# baseline (speedup 1.0000x reference)
"""Trainium2 Bass kernel for the nn_Decoder problem.

Math facts exploited (all exact in f32):
  - softmax over a singleton axis == 1.0, so betas == 1 and
    attn_weights == softmax(ones) == 1/L exactly (L = 2048 = 2^11).
  - Therefore c = relu(mean_L(encoder_out)) and the outputs do not depend
    on x / emb / attn_W / attn_b at all.

Sharding over 8 cores:
  - encoder_out batch-sharded (4 batches per core)       -> 32 MB/core
  - out_W vocab-sharded (4000 rows/core, natural layout) -> 16 MB/core
  - GRU weights H-sharded: core m owns columns [64m,64m+64) of h_f/h_b,
    i.e. rows {64m..} of each r/z/n block of W_ih/W_hh (pre-transposed so
    the contraction dim lands on partitions)
  - h replicated (tiny), out_b vocab-sharded.

Device pipeline per core (engine balance: DVE does the byte-heavy work,
PE only small matmuls — fp32 matmuls run as 2 HW passes and are slow):
  enc mean (DVE tree adds + PE ones-reduce) -> relu -> AllGather c ->
  DVE 32x32 transposes of c -> GRU matmuls + gates -> AllGather h31 ->
  broadcast cvec -> DVE mul+reduce over vocab shard (+bias) ->
  AllGather logits -> log_softmax in a [128, 250] layout -> logp.
"""

import numpy as np

import concourse.bacc as bacc
import concourse.bass as bass
import concourse.tile as tile
from concourse import bass_isa, mybir
from concourse.bass_utils import run_bass_kernel_spmd

F32 = mybir.dt.float32
BF16 = mybir.dt.bfloat16
OW_BF16 = True          # out-projection weights in bf16 (logp absmax ~1e-2, rel ~7e-4)
OWDT = BF16 if OW_BF16 else F32

B, L, E, H, V = 32, 2048, 512, 512, 32000
D2 = 2 * H          # 1024, GRU input dim / encoder feature dim
NC = 8              # cores
BP = B // NC        # 4 batches per core
HS = H // NC        # 64 H columns per core
VS = V // NC        # 4000 vocab rows per core
G = 3 * HS          # 192 gate rows per core
SUP = 4             # L-chunks of 128 per enc supertile
NSUP = L // (128 * SUP)   # 4 supertiles per batch
NVT = (VS + 127) // 128   # 32 vocab tiles per core (last one is 32 rows)


def _build_program():
    nc = bacc.Bacc("TRN2", target_bir_lowering=False, debug=False, num_devices=NC)

    # ---- I/O ----
    enc = nc.dram_tensor("enc", [BP, L, D2], F32, kind="ExternalInput")
    wtif = nc.dram_tensor("wtif", [D2, G], F32, kind="ExternalInput")
    wthf = nc.dram_tensor("wthf", [H, G], F32, kind="ExternalInput")
    wtib = nc.dram_tensor("wtib", [D2, G], F32, kind="ExternalInput")
    wthb = nc.dram_tensor("wthb", [H, G], F32, kind="ExternalInput")
    bif = nc.dram_tensor("bif", [1, G], F32, kind="ExternalInput")
    bhf = nc.dram_tensor("bhf", [1, G], F32, kind="ExternalInput")
    bib = nc.dram_tensor("bib", [1, G], F32, kind="ExternalInput")
    bhb = nc.dram_tensor("bhb", [1, G], F32, kind="ExternalInput")
    h0t = nc.dram_tensor("h0t", [H, B], F32, kind="ExternalInput")
    h1t = nc.dram_tensor("h1t", [H, B], F32, kind="ExternalInput")
    h0s = nc.dram_tensor("h0s", [B, HS], F32, kind="ExternalInput")
    h1s = nc.dram_tensor("h1s", [B, HS], F32, kind="ExternalInput")
    owt = nc.dram_tensor("owt", [D2, VS], OWDT, kind="ExternalInput")
    ob = nc.dram_tensor("ob", [1, VS], OWDT, kind="ExternalInput")

    hf_o = nc.dram_tensor("hf", [B, HS], F32, kind="ExternalOutput")
    hb_o = nc.dram_tensor("hb", [B, HS], F32, kind="ExternalOutput")
    logp_o = nc.dram_tensor("logp", [V], F32, kind="ExternalOutput")
    att_o = nc.dram_tensor("att", [BP, L], F32, kind="ExternalOutput")

    grp = [list(range(NC))]

    with tile.TileContext(nc) as tc:
        with (
            tc.tile_pool(name="const", bufs=1) as cp,
            tc.tile_pool(name="dram", bufs=1, space="DRAM") as dp,
            tc.tile_pool(name="encp", bufs=3) as encp,
            tc.tile_pool(name="owp", bufs=6) as owp,
            tc.tile_pool(name="accp", bufs=1) as accp,
            tc.tile_pool(name="work", bufs=2) as wp,
            tc.tile_pool(name="gates", bufs=2) as gp,
            tc.tile_pool(name="soft", bufs=1) as sp,
        ):
            # ---- DRAM bounce buffers ----
            warm_in = dp.tile([2], F32, tag="warm_in")
            warm_out = dp.tile([2 * NC], F32, tag="warm_out")
            c_part = [dp.tile([D2], F32, tag=f"c_part{b}", name=f"c_part{b}")
                      for b in range(BP)]
            c_all = [dp.tile([NC * D2], F32, tag=f"c_all{b}", name=f"c_all{b}")
                     for b in range(BP)]
            b31 = dp.tile([2 * HS], F32, tag="b31")
            g31 = dp.tile([2 * H], F32, tag="g31")
            lg = dp.tile([VS], F32, tag="lg")
            lgfull = dp.tile([V], F32, tag="lgfull")

            # ---- warm up the collectives path while enc streams ----
            wtile = cp.tile([1, 2], F32, tag="wtile")
            nc.vector.memset(wtile, 0.0)
            nc.scalar.dma_start(out=warm_in[:], in_=wtile[0:1, :])
            nc.gpsimd.collective_compute(
                "AllGather", mybir.AluOpType.bypass, replica_groups=grp,
                ins=[warm_in[:].opt()], outs=[warm_out[:].opt()],
            )

            # preload ACT function tables (first use costs ~1.3us each)
            dumA = cp.tile([1, 1], F32, tag="dumA")
            dumB = cp.tile([1, 1], F32, tag="dumB")
            nc.vector.memset(dumA, 1.0)
            for fn in (mybir.ActivationFunctionType.Relu,
                       mybir.ActivationFunctionType.Sigmoid,
                       mybir.ActivationFunctionType.Tanh,
                       mybir.ActivationFunctionType.Exp,
                       mybir.ActivationFunctionType.Ln):
                nc.scalar.activation(out=dumB, in_=dumA, func=fn)

            # ---- constants / weights to SBUF ----
            ones128 = cp.tile([128, 1], F32, tag="ones128")
            nc.vector.memset(ones128, 1.0)
            ones1 = cp.tile([1, 32], F32, tag="ones1")
            nc.vector.memset(ones1, 1.0)
            att_sb = cp.tile([B, HS], F32, tag="attc")
            nc.vector.memset(att_sb, 1.0 / L)

            wtif_sb = cp.tile([128, 8, G], F32, tag="wtif")
            nc.sync.dma_start(out=wtif_sb, in_=wtif.rearrange("(k p) j -> p k j", p=128))
            wtib_sb = cp.tile([128, 8, G], F32, tag="wtib")
            nc.sync.dma_start(out=wtib_sb, in_=wtib.rearrange("(k p) j -> p k j", p=128))
            wthf_sb = cp.tile([128, 4, G], F32, tag="wthf")
            nc.sync.dma_start(out=wthf_sb, in_=wthf.rearrange("(k p) j -> p k j", p=128))
            wthb_sb = cp.tile([128, 4, G], F32, tag="wthb")
            nc.sync.dma_start(out=wthb_sb, in_=wthb.rearrange("(k p) j -> p k j", p=128))
            h0t_sb = cp.tile([128, 4, B], F32, tag="h0t")
            nc.sync.dma_start(out=h0t_sb, in_=h0t.rearrange("(k p) b -> p k b", p=128))
            h1t_sb = cp.tile([128, 4, B], F32, tag="h1t")
            nc.sync.dma_start(out=h1t_sb, in_=h1t.rearrange("(k p) b -> p k b", p=128))
            h0s_sb = cp.tile([B, HS], F32, tag="h0s")
            nc.sync.dma_start(out=h0s_sb, in_=h0s[:, :])
            h1s_sb = cp.tile([B, HS], F32, tag="h1s")
            nc.sync.dma_start(out=h1s_sb, in_=h1s[:, :])
            bias_sb = {}
            for nm, t in (("bif", bif), ("bhf", bhf), ("bib", bib), ("bhb", bhb)):
                bias_sb[nm] = cp.tile([1, G], F32, tag=nm, name=nm)
                nc.sync.dma_start(out=bias_sb[nm], in_=t[:, :])
            ones1b = cp.tile([1, 1], OWDT, tag="ones1b")
            nc.vector.memset(ones1b, 1.0)
            ob_sb = cp.tile([1, VS], OWDT, tag="ob")
            nc.sync.dma_start(out=ob_sb, in_=ob[:, :])

            # attn_weights output: exact constant 1/L
            for b in range(BP):
                nc.scalar.dma_start(
                    out=att_o[b].rearrange("(p f) -> p f", f=HS),
                    in_=att_sb,
                )

            # ---- phase 1: mean over L of encoder_out ----
            # batch 0 reduces on PE (ones-matmul), batches 1-2 on DVE,
            # batch 3 on GpSimd: three parallel lanes, each ~35us, so the
            # phase is DMA-bound. Each batch's c is AllGathered as soon as
            # it is ready, hiding the collective latency under the stream.
            acc = [None] + [accp.tile([128, D2], F32, tag=f"acc{b}", name=f"acc{b}")
                            for b in range(1, BP)]
            with tc.tile_pool(name="mps", bufs=1, space="PSUM") as mps:
                ps0 = mps.tile([1, D2], F32, tag="mps0", name="mps0")
                for b in range(BP):
                    eng = nc.gpsimd if b == 3 else nc.vector
                    for s in range(NSUP):
                        st = encp.tile([128, SUP, D2], F32, tag="enc")
                        nc.sync.dma_start(
                            out=st,
                            in_=enc[b, 128 * SUP * s:128 * SUP * (s + 1), :].rearrange(
                                "(t p) d -> p t d", p=128
                            ),
                        )
                        if b == 0:
                            for t in range(SUP):
                                for hh in range(2):
                                    nc.tensor.matmul(
                                        ps0[0:1, 512 * hh:512 * (hh + 1)],
                                        ones128,
                                        st[:, t, 512 * hh:512 * (hh + 1)],
                                        start=(s == 0 and t == 0),
                                        stop=(s == NSUP - 1 and t == SUP - 1),
                                    )
                        else:
                            if s == 0:
                                eng.tensor_add(acc[b], st[:, 0, :], st[:, 1, :])
                            else:
                                eng.tensor_add(acc[b], acc[b], st[:, 0, :])
                                eng.tensor_add(acc[b], acc[b], st[:, 1, :])
                            eng.tensor_add(acc[b], acc[b], st[:, 2, :])
                            eng.tensor_add(acc[b], acc[b], st[:, 3, :])
                    if b == 0:
                        c_b = wp.tile([1, D2], F32, tag="c_b")
                        nc.scalar.activation(
                            out=c_b, in_=ps0[0:1, :],
                            func=mybir.ActivationFunctionType.Relu,
                            scale=1.0 / L,
                        )
                        nc.scalar.dma_start(out=c_part[b][:], in_=c_b[0:1, :])
                    else:
                        ps = mps.tile([1, D2], F32, tag=f"mps{b}", name=f"mps{b}")
                        for hh in range(2):
                            nc.tensor.matmul(
                                ps[0:1, 512 * hh:512 * (hh + 1)],
                                ones128,
                                acc[b][:, 512 * hh:512 * (hh + 1)],
                                start=True, stop=True,
                            )
                        c_b = wp.tile([1, D2], F32, tag="c_b")
                        nc.scalar.activation(
                            out=c_b, in_=ps[0:1, :],
                            func=mybir.ActivationFunctionType.Relu,
                            scale=1.0 / L,
                        )
                        nc.scalar.dma_start(out=c_part[b][:], in_=c_b[0:1, :])
                    nc.gpsimd.collective_compute(
                        "AllGather", mybir.AluOpType.bypass, replica_groups=grp,
                        ins=[c_part[b][:].opt()], outs=[c_all[b][:].opt()],
                    )

            # ---- out_W^T shard streaming (behind enc loads on sync queue) ----
            owt_sb = []
            for k in range(8):
                ot = owp.tile([128, VS], OWDT, tag="owt", name=f"owt{k}")
                nc.sync.dma_start(out=ot, in_=owt[128 * k:128 * (k + 1), :])
                owt_sb.append(ot)

            # ---- phase 2: load gathered c, transpose, GRU ----
            # c_all[b] rows are per-core batch b -> global batch 4*j + b
            # c_full row r = 8*b + j holds global batch g = 4*j + b; the h
            # inputs are host-permuted to the same order and the h outputs are
            # written back through a permuted AP.
            c_full = cp.tile([B, D2], F32, tag="c_full")
            for b in range(BP):
                nc.scalar.dma_start(
                    out=c_full[NC * b:NC * (b + 1), :],
                    in_=c_all[b].rearrange("(j d) -> j d", j=NC),
                )
            cT = [cp.tile([128, 32], F32, tag=f"cT{k}", name=f"cT{k}")
                  for k in range(8)]
            for k in range(8):
                for j in range(4):
                    nc.vector.transpose(
                        out=cT[k][32 * j:32 * (j + 1), :],
                        in_=c_full[0:B, 128 * k + 32 * j:128 * k + 32 * (j + 1)],
                    )

            with tc.tile_pool(name="gps", bufs=1, space="PSUM") as gps:
                for cell, (wti_sb, wth_sb, bi, bh, ht_sb, hs_sb, h_out, boff) in enumerate((
                    (wtif_sb, wthf_sb, bias_sb["bif"], bias_sb["bhf"], h0t_sb, h0s_sb, hf_o, 0),
                    (wtib_sb, wthb_sb, bias_sb["bib"], bias_sb["bhb"], h1t_sb, h1s_sb, hb_o, HS),
                )):
                    # r/z pre-activations: gi_rz + gh_rz + biases accumulated
                    # in ONE psum region (a DVE op may read only one PSUM input)
                    rzp = gps.tile([B, 2 * HS], F32, tag=f"rzp{cell}", name=f"rzp{cell}")
                    for k in range(8):
                        nc.tensor.matmul(
                            rzp, cT[k], wti_sb[:, k, 0:2 * HS],
                            start=(k == 0), stop=False,
                        )
                    for k in range(4):
                        nc.tensor.matmul(
                            rzp, ht_sb[:, k, :], wth_sb[:, k, 0:2 * HS],
                            start=False, stop=False,
                        )
                    nc.tensor.matmul(rzp, ones1, bi[0:1, 0:2 * HS], start=False, stop=False)
                    nc.tensor.matmul(rzp, ones1, bh[0:1, 0:2 * HS], start=False, stop=True)
                    # n-gate halves kept separate: n = tanh(gin + r * ghn)
                    gin = gps.tile([B, HS], F32, tag=f"gin{cell}", name=f"gin{cell}")
                    for k in range(8):
                        nc.tensor.matmul(
                            gin, cT[k], wti_sb[:, k, 2 * HS:G],
                            start=(k == 0), stop=False,
                        )
                    nc.tensor.matmul(gin, ones1, bi[0:1, 2 * HS:G], start=False, stop=True)
                    ghn = gps.tile([B, HS], F32, tag=f"ghn{cell}", name=f"ghn{cell}")
                    for k in range(4):
                        nc.tensor.matmul(
                            ghn, ht_sb[:, k, :], wth_sb[:, k, 2 * HS:G],
                            start=(k == 0), stop=False,
                        )
                    nc.tensor.matmul(ghn, ones1, bh[0:1, 2 * HS:G], start=False, stop=True)

                    rz = gp.tile([B, 2 * HS], F32, tag="rz")
                    nc.scalar.activation(
                        out=rz, in_=rzp, func=mybir.ActivationFunctionType.Sigmoid
                    )
                    t1 = gp.tile([B, HS], F32, tag="t1")
                    nc.vector.tensor_mul(t1, rz[:, 0:HS], ghn)
                    nin = gp.tile([B, HS], F32, tag="nin")
                    nc.vector.tensor_add(nin, t1, gin)
                    nt = gp.tile([B, HS], F32, tag="nt")
                    nc.scalar.activation(
                        out=nt, in_=nin, func=mybir.ActivationFunctionType.Tanh
                    )
                    # h' = n + z * (h_prev - n)
                    dt_ = gp.tile([B, HS], F32, tag="dt")
                    nc.vector.tensor_sub(dt_, hs_sb, nt)
                    mt = gp.tile([B, HS], F32, tag="mt")
                    nc.vector.tensor_mul(mt, rz[:, HS:2 * HS], dt_)
                    hcell = gp.tile([B, HS], F32, tag="hcell")
                    nc.vector.tensor_add(hcell, nt, mt)

                    hov = h_out.rearrange("(j four) c -> four j c", four=BP)
                    for b in range(BP):
                        nc.scalar.dma_start(
                            out=hov[b], in_=hcell[NC * b:NC * (b + 1), :]
                        )
                    nc.scalar.dma_start(out=b31[boff:boff + HS], in_=hcell[31:32, :])

            # ---- phase 3: AllGather h31, broadcast cvec, DVE matvec ----
            nc.gpsimd.collective_compute(
                "AllGather", mybir.AluOpType.bypass, replica_groups=grp,
                ins=[b31[:].opt()], outs=[g31[:].opt()],
            )
            # cvec[p, k] = cat(h_f31, h_b31)[128k + p], from interleaved g31:
            # cvec[512*t + 64*m + s] = g31[128*m + 64*t + s]
            cvec = cp.tile([128, 8], F32, tag="cvec")
            g31r = g31.rearrange("(kk ph t pl) -> ph t pl kk", kk=4, ph=2, t=2, pl=64)
            for ph in range(2):
                for t in range(2):
                    nc.scalar.dma_start(
                        out=cvec[64 * ph:64 * (ph + 1), 4 * t:4 * (t + 1)],
                        in_=g31r[ph, t],
                    )
            if OW_BF16:
                cvb = cp.tile([128, 8], BF16, tag="cvb")
                nc.vector.tensor_copy(cvb, cvec)
            else:
                cvb = cvec

            NS = VS // 8  # 500
            with tc.tile_pool(name="lgp", bufs=1, space="PSUM") as lgp:
                lg_ps = [lgp.tile([1, NS], F32, tag=f"lg{n}", name=f"lgps{n}")
                         for n in range(8)]
                for k in range(8):
                    for n in range(8):
                        nc.tensor.matmul(
                            lg_ps[n], cvb[:, k:k + 1],
                            owt_sb[k][:, NS * n:NS * (n + 1)],
                            start=(k == 0), stop=False,
                        )
                for n in range(8):
                    nc.tensor.matmul(
                        lg_ps[n], ones1b, ob_sb[0:1, NS * n:NS * (n + 1)],
                        start=False, stop=True,
                    )
                for n in range(8):
                    lgt = wp.tile([1, NS], F32, tag="lgt")
                    if n % 2 == 0:
                        nc.scalar.copy(lgt, lg_ps[n])
                    else:
                        nc.vector.tensor_copy(lgt, lg_ps[n])
                    nc.scalar.dma_start(out=lg[NS * n:NS * (n + 1)], in_=lgt[0:1, :])

            # ---- phase 4: AllGather logits, log_softmax ----
            nc.gpsimd.collective_compute(
                "AllGather", mybir.AluOpType.bypass, replica_groups=grp,
                ins=[lg[:].opt()], outs=[lgfull[:].opt()],
            )
            FP = V // 128  # 250
            ls = sp.tile([128, FP], F32, tag="ls")
            nc.scalar.dma_start(out=ls, in_=lgfull.rearrange("(p f) -> p f", p=128))
            mx = sp.tile([128, 1], F32, tag="mx")
            nc.vector.tensor_reduce(
                out=mx, in_=ls, axis=mybir.AxisListType.X, op=mybir.AluOpType.max
            )
            mxr = sp.tile([128, 1], F32, tag="mxr")
            nc.gpsimd.partition_all_reduce(mxr, mx, 128, bass_isa.ReduceOp.max)
            nmx = sp.tile([128, 1], F32, tag="nmx")
            nc.vector.tensor_scalar_mul(nmx, mxr, -1.0)
            ex = sp.tile([128, FP], F32, tag="ex")
            sm = sp.tile([128, 1], F32, tag="sm")
            nc.scalar.activation(
                out=ex, in_=ls, func=mybir.ActivationFunctionType.Exp,
                bias=nmx[:, 0:1], scale=1.0, accum_out=sm[:, 0:1],
            )
            smr = sp.tile([128, 1], F32, tag="smr")
            nc.gpsimd.partition_all_reduce(smr, sm, 128, bass_isa.ReduceOp.add)
            lnS = sp.tile([128, 1], F32, tag="lnS")
            nc.scalar.activation(
                out=lnS, in_=smr, func=mybir.ActivationFunctionType.Ln
            )
            lse = sp.tile([128, 1], F32, tag="lse")
            nc.vector.tensor_add(lse, mxr, lnS)
            nlse = sp.tile([128, 1], F32, tag="nlse")
            nc.vector.tensor_scalar_mul(nlse, lse, -1.0)
            lp = sp.tile([128, FP], F32, tag="lp")
            nc.scalar.activation(
                out=lp, in_=ls, func=mybir.ActivationFunctionType.Identity,
                bias=nlse[:, 0:1], scale=1.0,
            )
            nc.scalar.dma_start(
                out=logp_o.rearrange("(p f) -> p f", p=128), in_=lp
            )

    nc.compile()
    return nc


_NC_CACHE = None


def get_nc():
    global _NC_CACHE
    if _NC_CACHE is None:
        _NC_CACHE = _build_program()
    return _NC_CACHE


def prepare_in_maps(inputs):
    """Shard full inputs into the 8 per-core input dicts."""
    f = np.float32
    enc = np.ascontiguousarray(np.asarray(inputs["encoder_out"], dtype=f))
    h = np.asarray(inputs["h"], dtype=f)
    W_ih_f = np.asarray(inputs["W_ih_f"], dtype=f)
    W_hh_f = np.asarray(inputs["W_hh_f"], dtype=f)
    b_ih_f = np.asarray(inputs["b_ih_f"], dtype=f)
    b_hh_f = np.asarray(inputs["b_hh_f"], dtype=f)
    W_ih_b = np.asarray(inputs["W_ih_b"], dtype=f)
    W_hh_b = np.asarray(inputs["W_hh_b"], dtype=f)
    b_ih_b = np.asarray(inputs["b_ih_b"], dtype=f)
    b_hh_b = np.asarray(inputs["b_hh_b"], dtype=f)
    out_W = np.ascontiguousarray(np.asarray(inputs["out_W"], dtype=f))
    out_b = np.asarray(inputs["out_b"], dtype=f)

    # device batch order r=8b+j <-> global batch 4j+b
    order = np.array([4 * (r % 8) + r // 8 for r in range(B)])
    h0t = np.ascontiguousarray(h[0].T[:, order])
    h1t = np.ascontiguousarray(h[1].T[:, order])
    import ml_dtypes
    owdt = ml_dtypes.bfloat16 if OW_BF16 else np.float32

    in_maps = []
    for m in range(NC):
        rows = np.r_[HS * m:HS * (m + 1),
                     H + HS * m:H + HS * (m + 1),
                     2 * H + HS * m:2 * H + HS * (m + 1)]
        in_maps.append({
            "enc": enc[BP * m:BP * (m + 1)],
            "wtif": np.ascontiguousarray(W_ih_f[rows, :].T),
            "wthf": np.ascontiguousarray(W_hh_f[rows, :].T),
            "wtib": np.ascontiguousarray(W_ih_b[rows, :].T),
            "wthb": np.ascontiguousarray(W_hh_b[rows, :].T),
            "bif": np.ascontiguousarray(b_ih_f[rows][None, :]),
            "bhf": np.ascontiguousarray(b_hh_f[rows][None, :]),
            "bib": np.ascontiguousarray(b_ih_b[rows][None, :]),
            "bhb": np.ascontiguousarray(b_hh_b[rows][None, :]),
            "h0t": h0t,
            "h1t": h1t,
            "h0s": np.ascontiguousarray(h[0][order][:, HS * m:HS * (m + 1)]),
            "h1s": np.ascontiguousarray(h[1][order][:, HS * m:HS * (m + 1)]),
            "owt": np.ascontiguousarray(out_W[VS * m:VS * (m + 1), :].T.astype(owdt)),
            "ob": np.ascontiguousarray(out_b[VS * m:VS * (m + 1)].reshape(1, VS).astype(owdt)),
        })
    return in_maps


def assemble(results):
    """Combine per-core outputs into the reference's (logp, h_new, attn)."""
    hf = np.concatenate([results[m]["hf"] for m in range(NC)], axis=1)
    hb = np.concatenate([results[m]["hb"] for m in range(NC)], axis=1)
    h_new = np.stack([hf, hb], axis=0)
    logp = results[0]["logp"].reshape(1, V)
    attn = np.concatenate([results[m]["att"] for m in range(NC)], axis=0)
    return logp, h_new, attn


def kernel(**inputs):
    nc = get_nc()
    in_maps = prepare_in_maps(inputs)
    res = run_bass_kernel_spmd(nc, in_maps, list(range(NC)))
    return assemble(res.results)


# revision 19
# speedup vs baseline: 1.6588x; 1.6588x over previous
"""Trainium2 Bass kernel for the nn_Decoder problem.

Math facts exploited (all exact in f32):
  - softmax over a singleton axis == 1.0, so betas == 1 and
    attn_weights == softmax(ones) == 1/L exactly (L = 2048 = 2^11).
  - Therefore c = relu(mean_L(encoder_out)) and the outputs do not depend
    on x / emb / attn_W / attn_b at all.

Sharding over 8 cores:
  - encoder_out batch-sharded (4 batches per core)       -> 32 MB/core
  - out_W vocab-sharded (4000 rows/core, natural layout) -> 16 MB/core
  - GRU weights H-sharded: core m owns columns [64m,64m+64) of h_f/h_b,
    i.e. rows {64m..} of each r/z/n block of W_ih/W_hh (pre-transposed so
    the contraction dim lands on partitions)
  - h replicated (tiny), out_b vocab-sharded.

Device pipeline per core (engine balance: DVE does the byte-heavy work,
PE only small matmuls — fp32 matmuls run as 2 HW passes and are slow):
  enc mean (DVE tree adds + PE ones-reduce) -> relu -> AllGather c ->
  DVE 32x32 transposes of c -> GRU matmuls + gates -> AllGather h31 ->
  broadcast cvec -> DVE mul+reduce over vocab shard (+bias) ->
  AllGather logits -> log_softmax in a [128, 250] layout -> logp.
"""

import numpy as np

import concourse.bacc as bacc
import concourse.bass as bass
import concourse.tile as tile
from concourse import bass_isa, mybir
from concourse.bass_utils import run_bass_kernel_spmd

F32 = mybir.dt.float32
BF16 = mybir.dt.bfloat16
OW_BF16 = True          # out-projection weights in bf16 (logp absmax ~1e-2, rel ~7e-4)
ENC_BF16 = True         # encoder_out in bf16, fp32 PSUM accumulate (h absmax ~1.4e-4)
OWDT = BF16 if OW_BF16 else F32
ENDT = BF16 if ENC_BF16 else F32

B, L, E, H, V = 32, 2048, 512, 512, 32000
D2 = 2 * H          # 1024, GRU input dim / encoder feature dim
NC = 8              # cores
BP = B // NC        # 4 batches per core
HS = H // NC        # 64 H columns per core
VS = V // NC        # 4000 vocab rows per core
G = 3 * HS          # 192 gate rows per core
SUP = 4             # L-chunks of 128 per enc supertile
NSUP = L // (128 * SUP)   # 4 supertiles per batch
NVT = (VS + 127) // 128   # 32 vocab tiles per core (last one is 32 rows)


def _build_program():
    nc = bacc.Bacc("TRN2", target_bir_lowering=False, debug=False, num_devices=NC)

    # ---- I/O ----
    enc = nc.dram_tensor("enc", [BP, L, D2], ENDT, kind="ExternalInput")
    wtif = nc.dram_tensor("wtif", [D2, G], F32, kind="ExternalInput")
    wthf = nc.dram_tensor("wthf", [H, G], F32, kind="ExternalInput")
    wtib = nc.dram_tensor("wtib", [D2, G], F32, kind="ExternalInput")
    wthb = nc.dram_tensor("wthb", [H, G], F32, kind="ExternalInput")
    bif = nc.dram_tensor("bif", [1, G], F32, kind="ExternalInput")
    bhf = nc.dram_tensor("bhf", [1, G], F32, kind="ExternalInput")
    bib = nc.dram_tensor("bib", [1, G], F32, kind="ExternalInput")
    bhb = nc.dram_tensor("bhb", [1, G], F32, kind="ExternalInput")
    h0t = nc.dram_tensor("h0t", [H, B], F32, kind="ExternalInput")
    h1t = nc.dram_tensor("h1t", [H, B], F32, kind="ExternalInput")
    h0s = nc.dram_tensor("h0s", [B, HS], F32, kind="ExternalInput")
    h1s = nc.dram_tensor("h1s", [B, HS], F32, kind="ExternalInput")
    owt = nc.dram_tensor("owt", [D2, VS], OWDT, kind="ExternalInput")
    ob = nc.dram_tensor("ob", [1, VS], OWDT, kind="ExternalInput")

    hf_o = nc.dram_tensor("hf", [B, HS], F32, kind="ExternalOutput")
    hb_o = nc.dram_tensor("hb", [B, HS], F32, kind="ExternalOutput")
    logp_o = nc.dram_tensor("logp", [V], F32, kind="ExternalOutput")
    att_o = nc.dram_tensor("att", [BP, L], F32, kind="ExternalOutput")

    grp = [list(range(NC))]

    with tile.TileContext(nc) as tc:
        with (
            tc.tile_pool(name="const", bufs=1) as cp,
            tc.tile_pool(name="dram", bufs=1, space="DRAM") as dp,
            tc.tile_pool(name="encp", bufs=4) as encp,
            tc.tile_pool(name="owp", bufs=1) as owp,
            tc.tile_pool(name="work", bufs=2) as wp,
            tc.tile_pool(name="gates", bufs=2) as gp,
            tc.tile_pool(name="soft", bufs=1) as sp,
        ):
            # ---- DRAM bounce buffers ----
            warm_in = dp.tile([2], F32, tag="warm_in")
            warm_out = dp.tile([2 * NC], F32, tag="warm_out")
            c_part = dp.tile([BP * D2], F32, tag="c_part")
            c_all = dp.tile([B * D2], F32, tag="c_all")
            b31 = dp.tile([2 * HS], F32, tag="b31")
            g31 = dp.tile([2 * H], F32, tag="g31")
            lg = dp.tile([VS], F32, tag="lg")
            lgfull = dp.tile([V], F32, tag="lgfull")

            # ---- warm up the collectives path while enc streams ----
            wtile = cp.tile([1, 2], F32, tag="wtile")
            nc.vector.memset(wtile, 0.0)
            nc.scalar.dma_start(out=warm_in[:], in_=wtile[0:1, :])
            nc.gpsimd.collective_compute(
                "AllGather", mybir.AluOpType.bypass, replica_groups=grp,
                ins=[warm_in[:].opt()], outs=[warm_out[:].opt()],
            )

            # preload ACT function tables (first use costs ~1.3us each)
            dumA = cp.tile([1, 1], F32, tag="dumA")
            dumB = cp.tile([1, 1], F32, tag="dumB")
            nc.vector.memset(dumA, 1.0)
            for fn in (mybir.ActivationFunctionType.Relu,
                       mybir.ActivationFunctionType.Sigmoid,
                       mybir.ActivationFunctionType.Tanh,
                       mybir.ActivationFunctionType.Exp,
                       mybir.ActivationFunctionType.Ln):
                nc.scalar.activation(out=dumB, in_=dumA, func=fn)

            # ---- constants / weights to SBUF ----
            ones128 = cp.tile([128, 1], ENDT, tag="ones128")
            nc.vector.memset(ones128, 1.0)
            ones1 = cp.tile([1, 32], F32, tag="ones1")
            nc.vector.memset(ones1, 1.0)
            att_sb = cp.tile([B, HS], F32, tag="attc")
            nc.vector.memset(att_sb, 1.0 / L)

            wtif_sb = cp.tile([128, 8, G], F32, tag="wtif")
            nc.sync.dma_start(out=wtif_sb, in_=wtif.rearrange("(k p) j -> p k j", p=128))
            wtib_sb = cp.tile([128, 8, G], F32, tag="wtib")
            nc.sync.dma_start(out=wtib_sb, in_=wtib.rearrange("(k p) j -> p k j", p=128))
            wthf_sb = cp.tile([128, 4, G], F32, tag="wthf")
            nc.sync.dma_start(out=wthf_sb, in_=wthf.rearrange("(k p) j -> p k j", p=128))
            wthb_sb = cp.tile([128, 4, G], F32, tag="wthb")
            nc.sync.dma_start(out=wthb_sb, in_=wthb.rearrange("(k p) j -> p k j", p=128))
            h0t_sb = cp.tile([128, 4, B], F32, tag="h0t")
            nc.sync.dma_start(out=h0t_sb, in_=h0t.rearrange("(k p) b -> p k b", p=128))
            h1t_sb = cp.tile([128, 4, B], F32, tag="h1t")
            nc.sync.dma_start(out=h1t_sb, in_=h1t.rearrange("(k p) b -> p k b", p=128))
            h0s_sb = cp.tile([B, HS], F32, tag="h0s")
            nc.sync.dma_start(out=h0s_sb, in_=h0s[:, :])
            h1s_sb = cp.tile([B, HS], F32, tag="h1s")
            nc.sync.dma_start(out=h1s_sb, in_=h1s[:, :])
            bias_sb = {}
            for nm, t in (("bif", bif), ("bhf", bhf), ("bib", bib), ("bhb", bhb)):
                bias_sb[nm] = cp.tile([1, G], F32, tag=nm, name=nm)
                nc.sync.dma_start(out=bias_sb[nm], in_=t[:, :])
            ones1b = cp.tile([1, 1], OWDT, tag="ones1b")
            nc.vector.memset(ones1b, 1.0)
            ob_sb = cp.tile([1, VS], OWDT, tag="ob")
            nc.sync.dma_start(out=ob_sb, in_=ob[:, :])

            # attn_weights output: exact constant 1/L
            for b in range(BP):
                nc.scalar.dma_start(
                    out=att_o[b].rearrange("(p f) -> p f", f=HS),
                    in_=att_sb,
                )

            # ---- phase 1: mean over L of encoder_out (PE ones-matmul) ----
            # bf16 moving operand -> single-pass matmuls; fp32 PSUM gives the
            # exact sum of the (quantized) inputs.
            with tc.tile_pool(name="mps", bufs=1, space="PSUM") as mps:
                for b in range(BP):
                    ps = mps.tile([1, D2], F32, tag=f"mps{b}", name=f"mps{b}")
                    for s in range(NSUP):
                        st = encp.tile([128, SUP, D2], ENDT, tag="enc")
                        nc.sync.dma_start(
                            out=st,
                            in_=enc[b, 128 * SUP * s:128 * SUP * (s + 1), :].rearrange(
                                "(t p) d -> p t d", p=128
                            ),
                        )
                        for t in range(SUP):
                            for hh in range(2):
                                nc.tensor.matmul(
                                    ps[0:1, 512 * hh:512 * (hh + 1)],
                                    ones128,
                                    st[:, t, 512 * hh:512 * (hh + 1)],
                                    start=(s == 0 and t == 0),
                                    stop=(s == NSUP - 1 and t == SUP - 1),
                                )
                    c_b = wp.tile([1, D2], F32, tag="c_b")
                    nc.scalar.activation(
                        out=c_b, in_=ps[0:1, :],
                        func=mybir.ActivationFunctionType.Relu,
                        scale=1.0 / L,
                    )
                    nc.scalar.dma_start(out=c_part[D2 * b:D2 * (b + 1)], in_=c_b[0:1, :])

            nc.gpsimd.collective_compute(
                "AllGather", mybir.AluOpType.bypass, replica_groups=grp,
                ins=[c_part[:].opt()], outs=[c_all[:].opt()],
            )

            # ---- phase 2: load gathered c, transpose; owt streams behind ----
            c_full = cp.tile([B, D2], F32, tag="c_full")
            nc.scalar.dma_start(out=c_full, in_=c_all.rearrange("(b d) -> b d", b=B))
            # out_W^T loads ride the same ACT HWDGE ring AFTER the c_full
            # load, so they cannot starve the c AllGather of fabric bandwidth.
            owt_sb = []
            for k in range(8):
                ot = owp.tile([128, VS], OWDT, tag=f"owt{k}", name=f"owt{k}")
                nc.scalar.dma_start(out=ot, in_=owt[128 * k:128 * (k + 1), :])
                owt_sb.append(ot)
            cT = [cp.tile([128, 32], F32, tag=f"cT{k}", name=f"cT{k}")
                  for k in range(8)]
            for k in range(8):
                for j in range(4):
                    nc.vector.transpose(
                        out=cT[k][32 * j:32 * (j + 1), :],
                        in_=c_full[0:B, 128 * k + 32 * j:128 * k + 32 * (j + 1)],
                    )

            with tc.tile_pool(name="gps", bufs=1, space="PSUM") as gps:
                for cell, (wti_sb, wth_sb, bi, bh, ht_sb, hs_sb, h_out, boff) in enumerate((
                    (wtif_sb, wthf_sb, bias_sb["bif"], bias_sb["bhf"], h0t_sb, h0s_sb, hf_o, 0),
                    (wtib_sb, wthb_sb, bias_sb["bib"], bias_sb["bhb"], h1t_sb, h1s_sb, hb_o, HS),
                )):
                    # r/z pre-activations: gi_rz + gh_rz + biases accumulated
                    # in ONE psum region (a DVE op may read only one PSUM input)
                    rzp = gps.tile([B, 2 * HS], F32, tag=f"rzp{cell}", name=f"rzp{cell}")
                    for k in range(8):
                        nc.tensor.matmul(
                            rzp, cT[k], wti_sb[:, k, 0:2 * HS],
                            start=(k == 0), stop=False,
                        )
                    for k in range(4):
                        nc.tensor.matmul(
                            rzp, ht_sb[:, k, :], wth_sb[:, k, 0:2 * HS],
                            start=False, stop=False,
                        )
                    nc.tensor.matmul(rzp, ones1, bi[0:1, 0:2 * HS], start=False, stop=False)
                    nc.tensor.matmul(rzp, ones1, bh[0:1, 0:2 * HS], start=False, stop=True)
                    # n-gate halves kept separate: n = tanh(gin + r * ghn)
                    gin = gps.tile([B, HS], F32, tag=f"gin{cell}", name=f"gin{cell}")
                    for k in range(8):
                        nc.tensor.matmul(
                            gin, cT[k], wti_sb[:, k, 2 * HS:G],
                            start=(k == 0), stop=False,
                        )
                    nc.tensor.matmul(gin, ones1, bi[0:1, 2 * HS:G], start=False, stop=True)
                    ghn = gps.tile([B, HS], F32, tag=f"ghn{cell}", name=f"ghn{cell}")
                    for k in range(4):
                        nc.tensor.matmul(
                            ghn, ht_sb[:, k, :], wth_sb[:, k, 2 * HS:G],
                            start=(k == 0), stop=False,
                        )
                    nc.tensor.matmul(ghn, ones1, bh[0:1, 2 * HS:G], start=False, stop=True)

                    rz = gp.tile([B, 2 * HS], F32, tag="rz")
                    nc.scalar.activation(
                        out=rz, in_=rzp, func=mybir.ActivationFunctionType.Sigmoid
                    )
                    t1 = gp.tile([B, HS], F32, tag="t1")
                    nc.vector.tensor_mul(t1, rz[:, 0:HS], ghn)
                    nin = gp.tile([B, HS], F32, tag="nin")
                    nc.vector.tensor_add(nin, t1, gin)
                    nt = gp.tile([B, HS], F32, tag="nt")
                    nc.scalar.activation(
                        out=nt, in_=nin, func=mybir.ActivationFunctionType.Tanh
                    )
                    # h' = n + z * (h_prev - n)
                    dt_ = gp.tile([B, HS], F32, tag="dt")
                    nc.vector.tensor_sub(dt_, hs_sb, nt)
                    mt = gp.tile([B, HS], F32, tag="mt")
                    nc.vector.tensor_mul(mt, rz[:, HS:2 * HS], dt_)
                    hcell = gp.tile([B, HS], F32, tag="hcell")
                    nc.vector.tensor_add(hcell, nt, mt)

                    nc.sync.dma_start(out=h_out[:, :], in_=hcell)
                    nc.sync.dma_start(out=b31[boff:boff + HS], in_=hcell[31:32, :])

            # ---- phase 3: AllGather h31, broadcast cvec, DVE matvec ----
            nc.gpsimd.collective_compute(
                "AllGather", mybir.AluOpType.bypass, replica_groups=grp,
                ins=[b31[:].opt()], outs=[g31[:].opt()],
            )
            # cvec[p, k] = cat(h_f31, h_b31)[128k + p], from interleaved g31:
            # cvec[512*t + 64*m + s] = g31[128*m + 64*t + s]
            cvec = cp.tile([128, 8], F32, tag="cvec")
            g31r = g31.rearrange("(kk ph t pl) -> ph t pl kk", kk=4, ph=2, t=2, pl=64)
            for ph in range(2):
                for t in range(2):
                    nc.sync.dma_start(
                        out=cvec[64 * ph:64 * (ph + 1), 4 * t:4 * (t + 1)],
                        in_=g31r[ph, t],
                    )
            if OW_BF16:
                cvb = cp.tile([128, 8], BF16, tag="cvb")
                nc.vector.tensor_copy(cvb, cvec)
            else:
                cvb = cvec

            NS = VS // 8  # 500
            with tc.tile_pool(name="lgp", bufs=1, space="PSUM") as lgp:
                lg_ps = [lgp.tile([1, NS], F32, tag=f"lg{n}", name=f"lgps{n}")
                         for n in range(8)]
                for k in range(8):
                    for n in range(8):
                        nc.tensor.matmul(
                            lg_ps[n], cvb[:, k:k + 1],
                            owt_sb[k][:, NS * n:NS * (n + 1)],
                            start=(k == 0), stop=False,
                        )
                for n in range(8):
                    nc.tensor.matmul(
                        lg_ps[n], ones1b, ob_sb[0:1, NS * n:NS * (n + 1)],
                        start=False, stop=True,
                    )
                for n in range(8):
                    lgt = wp.tile([1, NS], F32, tag="lgt")
                    if n % 2 == 0:
                        nc.scalar.copy(lgt, lg_ps[n])
                    else:
                        nc.vector.tensor_copy(lgt, lg_ps[n])
                    nc.sync.dma_start(out=lg[NS * n:NS * (n + 1)], in_=lgt[0:1, :])

            # ---- phase 4: AllGather logits, log_softmax ----
            nc.gpsimd.collective_compute(
                "AllGather", mybir.AluOpType.bypass, replica_groups=grp,
                ins=[lg[:].opt()], outs=[lgfull[:].opt()],
            )
            FP = V // 128  # 250
            ls = sp.tile([128, FP], F32, tag="ls")
            nc.sync.dma_start(out=ls, in_=lgfull.rearrange("(p f) -> p f", p=128))
            mx = sp.tile([128, 1], F32, tag="mx")
            nc.vector.tensor_reduce(
                out=mx, in_=ls, axis=mybir.AxisListType.X, op=mybir.AluOpType.max
            )
            mxr = sp.tile([128, 1], F32, tag="mxr")
            nc.gpsimd.partition_all_reduce(mxr, mx, 128, bass_isa.ReduceOp.max)
            nmx = sp.tile([128, 1], F32, tag="nmx")
            nc.vector.tensor_scalar_mul(nmx, mxr, -1.0)
            ex = sp.tile([128, FP], F32, tag="ex")
            sm = sp.tile([128, 1], F32, tag="sm")
            nc.scalar.activation(
                out=ex, in_=ls, func=mybir.ActivationFunctionType.Exp,
                bias=nmx[:, 0:1], scale=1.0, accum_out=sm[:, 0:1],
            )
            smr = sp.tile([128, 1], F32, tag="smr")
            nc.gpsimd.partition_all_reduce(smr, sm, 128, bass_isa.ReduceOp.add)
            lnS = sp.tile([128, 1], F32, tag="lnS")
            nc.scalar.activation(
                out=lnS, in_=smr, func=mybir.ActivationFunctionType.Ln
            )
            lse = sp.tile([128, 1], F32, tag="lse")
            nc.vector.tensor_add(lse, mxr, lnS)
            nlse = sp.tile([128, 1], F32, tag="nlse")
            nc.vector.tensor_scalar_mul(nlse, lse, -1.0)
            lp = sp.tile([128, FP], F32, tag="lp")
            nc.scalar.activation(
                out=lp, in_=ls, func=mybir.ActivationFunctionType.Identity,
                bias=nlse[:, 0:1], scale=1.0,
            )
            nc.sync.dma_start(
                out=logp_o.rearrange("(p f) -> p f", p=128), in_=lp
            )

    nc.compile()
    return nc


_NC_CACHE = None


def get_nc():
    global _NC_CACHE
    if _NC_CACHE is None:
        _NC_CACHE = _build_program()
    return _NC_CACHE


def prepare_in_maps(inputs):
    """Shard full inputs into the 8 per-core input dicts."""
    f = np.float32
    enc = np.ascontiguousarray(np.asarray(inputs["encoder_out"], dtype=f))
    h = np.asarray(inputs["h"], dtype=f)
    W_ih_f = np.asarray(inputs["W_ih_f"], dtype=f)
    W_hh_f = np.asarray(inputs["W_hh_f"], dtype=f)
    b_ih_f = np.asarray(inputs["b_ih_f"], dtype=f)
    b_hh_f = np.asarray(inputs["b_hh_f"], dtype=f)
    W_ih_b = np.asarray(inputs["W_ih_b"], dtype=f)
    W_hh_b = np.asarray(inputs["W_hh_b"], dtype=f)
    b_ih_b = np.asarray(inputs["b_ih_b"], dtype=f)
    b_hh_b = np.asarray(inputs["b_hh_b"], dtype=f)
    out_W = np.ascontiguousarray(np.asarray(inputs["out_W"], dtype=f))
    out_b = np.asarray(inputs["out_b"], dtype=f)

    h0t = np.ascontiguousarray(h[0].T)
    h1t = np.ascontiguousarray(h[1].T)
    import ml_dtypes
    owdt = ml_dtypes.bfloat16 if OW_BF16 else np.float32
    endt = ml_dtypes.bfloat16 if ENC_BF16 else np.float32
    enc = np.ascontiguousarray(enc.astype(endt))

    in_maps = []
    for m in range(NC):
        rows = np.r_[HS * m:HS * (m + 1),
                     H + HS * m:H + HS * (m + 1),
                     2 * H + HS * m:2 * H + HS * (m + 1)]
        in_maps.append({
            "enc": enc[BP * m:BP * (m + 1)],
            "wtif": np.ascontiguousarray(W_ih_f[rows, :].T),
            "wthf": np.ascontiguousarray(W_hh_f[rows, :].T),
            "wtib": np.ascontiguousarray(W_ih_b[rows, :].T),
            "wthb": np.ascontiguousarray(W_hh_b[rows, :].T),
            "bif": np.ascontiguousarray(b_ih_f[rows][None, :]),
            "bhf": np.ascontiguousarray(b_hh_f[rows][None, :]),
            "bib": np.ascontiguousarray(b_ih_b[rows][None, :]),
            "bhb": np.ascontiguousarray(b_hh_b[rows][None, :]),
            "h0t": h0t,
            "h1t": h1t,
            "h0s": np.ascontiguousarray(h[0][:, HS * m:HS * (m + 1)]),
            "h1s": np.ascontiguousarray(h[1][:, HS * m:HS * (m + 1)]),
            "owt": np.ascontiguousarray(out_W[VS * m:VS * (m + 1), :].T.astype(owdt)),
            "ob": np.ascontiguousarray(out_b[VS * m:VS * (m + 1)].reshape(1, VS).astype(owdt)),
        })
    return in_maps


def assemble(results):
    """Combine per-core outputs into the reference's (logp, h_new, attn)."""
    hf = np.concatenate([results[m]["hf"] for m in range(NC)], axis=1)
    hb = np.concatenate([results[m]["hb"] for m in range(NC)], axis=1)
    h_new = np.stack([hf, hb], axis=0)
    logp = results[0]["logp"].reshape(1, V)
    attn = np.concatenate([results[m]["att"] for m in range(NC)], axis=0)
    return logp, h_new, attn


def kernel(**inputs):
    nc = get_nc()
    in_maps = prepare_in_maps(inputs)
    res = run_bass_kernel_spmd(nc, in_maps, list(range(NC)))
    return assemble(res.results)
